# revision 1
# baseline (speedup 1.0000x reference)
"""CenterLoss on 8 Trainium2 NeuronCores (Bass, raw engine programming).

Computes mean_i clip(||features_i - centers[labels_i,-1]||^2, 1e-12, 1e12)
for features [16384, 512] f32, labels [16384, 2] int, centers [10000, 512] f32.

Sharding: data-parallel over N. Each core gets 2048 rows of features and
their class ids; the centers table is replicated. On-device, each 128-row
tile gathers its matching center rows straight from DRAM via indirect DMA
(offsets = class ids), subtracts on VectorE, squares + row-reduces on
ScalarE (Square activation with accum_out; a few tiles on VectorE to
balance). The clamped per-row results are folded across partitions with a
ones-vector matmul on TensorE, so each core emits a single [1, 1] partial
sum; the host sums the 8 partials and divides by N.
"""

import sys

if "/opt/trn_rl_repo" not in sys.path:
    sys.path.insert(0, "/opt/trn_rl_repo")

import numpy as np

N, D, C = 16384, 512, 10000
N_CORES = 8
NS = N // N_CORES  # 2048 rows per core
P = 128
NT = NS // P  # 16 tiles of 128 rows per core
NBUF = 16  # feature/center tile buffers: all tiles resident, no reuse stalls
DBUF = 16  # diff tile buffers
CLAMP_MIN, CLAMP_MAX = 1e-12, 1e12

_cache = {}


def _build():
    from contextlib import ExitStack

    from concourse import bass, mybir

    nc = bass.Bass()
    feat = nc.dram_tensor("feat", [NS, D], mybir.dt.float32, kind="ExternalInput")
    idx = nc.dram_tensor("idx", [P, NT], mybir.dt.int32, kind="ExternalInput")
    cent = nc.dram_tensor("cent", [C, D], mybir.dt.float32, kind="ExternalInput")
    out = nc.dram_tensor("out", [1, 1], mybir.dt.float32, kind="ExternalOutput")

    with ExitStack() as ctx:
        idx_sb = ctx.enter_context(nc.sbuf_tensor([P, NT], mybir.dt.int32))
        fbuf = ctx.enter_context(nc.sbuf_tensor([P, NBUF, D], mybir.dt.float32))
        cbuf = ctx.enter_context(nc.sbuf_tensor([P, NBUF, D], mybir.dt.float32))
        dbuf = ctx.enter_context(nc.sbuf_tensor([P, DBUF, D], mybir.dt.float32))
        acc = ctx.enter_context(nc.sbuf_tensor([P, NT], mybir.dt.float32))
        acc2 = ctx.enter_context(nc.sbuf_tensor([P, NT], mybir.dt.float32))
        ones = ctx.enter_context(nc.sbuf_tensor([P, 1], mybir.dt.float32))
        red = ctx.enter_context(nc.sbuf_tensor([1, 1], mybir.dt.float32))
        ps = ctx.enter_context(nc.psum_tensor([1, NT], mybir.dt.float32))
        s_idx = ctx.enter_context(nc.semaphore("s_idx"))
        s_ones = ctx.enter_context(nc.semaphore("s_ones"))
        s_clamp = ctx.enter_context(nc.semaphore("s_clamp"))
        s_diff = ctx.enter_context(nc.semaphore("s_diff"))
        s_sq = ctx.enter_context(nc.semaphore("s_sq"))
        s_dsq = ctx.enter_context(nc.semaphore("s_dsq"))
        s_mm = ctx.enter_context(nc.semaphore("s_mm"))
        s_red = ctx.enter_context(nc.semaphore("s_red"))
        s_od = ctx.enter_context(nc.semaphore("s_od"))
        block = ctx.enter_context(nc.Block(no_gpsimd_drain=True))
        # Feature tiles are DMAed in groups (one big HWDGE DMA per group,
        # better efficiency than per-tile transfers); center gathers stay
        # one indirect DMA per tile - multi-row offset gathers miscompute
        # on hardware. One semaphore per DMA: a sem only ever has ONE
        # outstanding DMA (waiting for a partial count on a sem with
        # several outstanding DMAs is unsound - partial completions can sum
        # to the target without any single DMA being done). SWDGE (gather)
        # sems also can't be shared with HWDGE increments.
        GS = [5, 5, 5, 1]
        assert sum(GS) == NT
        g_start = [sum(GS[:k]) for k in range(len(GS))]
        tile_group = [k for k, n in enumerate(GS) for _ in range(n)]
        s_fd = [ctx.enter_context(nc.semaphore(f"s_fd{b}")) for b in range(len(GS))]
        # Gathers share sems in pairs (both inc; consumers wait for the full
        # 32, which is only reached when BOTH completed - waiting for a
        # partial count would be unsound). The last four tiles keep private
        # sems so the critical tail isn't gated on a neighbor's gather.
        gd_sem_of_tile = [t // 2 if t < 12 else t - 6 for t in range(NT)]
        gd_wait_val = [32 if t < 12 else 16 for t in range(NT)]
        n_gd = max(gd_sem_of_tile) + 1
        s_gd = [ctx.enter_context(nc.semaphore(f"s_gd{b}")) for b in range(n_gd)]

        # Which engine squares tile t: ScalarE (Square+accum) for most, DVE
        # (scalar_tensor_tensor with accum) for a few, to balance the two.
        DVE_SQ = {4, 9, 15}
        sq_owner = ["dve" if t in DVE_SQ else "act" for t in range(NT)]
        n_act = sq_owner.count("act")
        n_dve = sq_owner.count("dve")
        # cumulative counts: consumer progress needed to reuse dbuf slot of
        # tile u is "its square op completed"
        act_upto = [sum(1 for u in range(t + 1) if sq_owner[u] == "act") for t in range(NT)]
        dve_upto = [sum(1 for u in range(t + 1) if sq_owner[u] == "dve") for t in range(NT)]

        @block.sync
        def _(sync):
            for k, gn in enumerate(GS):
                t0 = g_start[k]
                # rows [t0*P, (t0+gn)*P) laid out as [P, gn, D] in SBUF
                src = feat[t0 * P : (t0 + gn) * P, :].rearrange(
                    "(g p) d -> p g d", p=P
                )
                sync.dma_start(out=fbuf[:, t0 : t0 + gn, :], in_=src).then_inc(
                    s_fd[k], 16
                )
            sync.wait_ge(s_red, 1)
            # single-partition 4B write: one descriptor. No completion wait:
            # the block-exit Sync drain quiesces the HWDGE queues, and the
            # postamble barrier leaves ~6us of margin past the ~1.7us receipt.
            sync.dma_start(out=out[:], in_=red[0:1, 0:1]).then_inc(s_od, 16)

        @block.gpsimd
        def _(gpsimd):
            gpsimd.wait_ge(s_idx, 16)
            for t in range(NT):
                gpsimd.indirect_dma_start(
                    out=cbuf[:, t, :],
                    out_offset=None,
                    in_=cent[:],
                    in_offset=bass.IndirectOffsetOnAxis(
                        ap=idx_sb[:, t : t + 1], axis=0
                    ),
                ).then_inc(s_gd[gd_sem_of_tile[t]], 16)
            # per-row d2 clamps (acc2 = min(max(acc, MIN), MAX)) run HERE on
            # the otherwise-idle GpSimd - on VectorE they'd serialize behind
            # the tail subtract/square ops. Staged: tiles 0..11 early, the
            # last 4 as soon as their squares land.
            gpsimd.wait_ge(s_sq, act_upto[11])
            gpsimd.wait_ge(s_dsq, dve_upto[11])
            gpsimd.tensor_scalar(
                out=acc2[:, :12],
                in0=acc[:, :12],
                scalar1=CLAMP_MIN,
                scalar2=CLAMP_MAX,
                op0=mybir.AluOpType.max,
                op1=mybir.AluOpType.min,
            ).then_inc(s_clamp, 1)
            gpsimd.wait_ge(s_sq, n_act)
            gpsimd.wait_ge(s_dsq, n_dve)
            gpsimd.tensor_scalar(
                out=acc2[:, 12:],
                in0=acc[:, 12:],
                scalar1=CLAMP_MIN,
                scalar2=CLAMP_MAX,
                op0=mybir.AluOpType.max,
                op1=mybir.AluOpType.min,
            ).then_inc(s_clamp, 1)

        @block.tensor
        def _(tensor):
            # cross-partition sums: ones[128,1]^T @ acc2 -> ps[1,:], staged so
            # most of the fold happens before the last tiles finish
            tensor.wait_ge(s_ones, 1)
            tensor.wait_ge(s_clamp, 1)
            tensor.matmul(
                out=ps[:, :12], lhsT=ones[:], rhs=acc2[:, :12], start=True, stop=True
            ).then_inc(s_mm, 1)
            tensor.wait_ge(s_clamp, 2)
            tensor.matmul(
                out=ps[:, 12:], lhsT=ones[:], rhs=acc2[:, 12:], start=True, stop=True
            ).then_inc(s_mm, 1)

        @block.vector
        def _(vector):
            vector.memset(ones[:], 1.0).then_inc(s_ones, 1)
            for t in range(NT):
                b = t % NBUF
                if t in g_start:
                    vector.wait_ge(s_fd[tile_group[t]], 16)
                vector.wait_ge(s_gd[gd_sem_of_tile[t]], gd_wait_val[t])
                if t >= DBUF:
                    u = t - DBUF  # dbuf slot is free once tile u was squared
                    if sq_owner[u] == "act":
                        vector.wait_ge(s_sq, act_upto[u])
                    else:
                        vector.wait_ge(s_dsq, dve_upto[u])
                vector.tensor_tensor(
                    out=dbuf[:, t % DBUF, :],
                    in0=fbuf[:, b, :],
                    in1=cbuf[:, b, :],
                    op=mybir.AluOpType.subtract,
                ).then_inc(s_diff, 1)
                if sq_owner[t] == "dve":
                    # square + row-sum on DVE for this tile (in place);
                    # self-wait: the sub above must clear the deep pipeline
                    vector.wait_ge(s_diff, t + 1)
                    vector.scalar_tensor_tensor(
                        out=dbuf[:, t % DBUF, :],
                        in0=dbuf[:, t % DBUF, :],
                        scalar=1.0,
                        in1=dbuf[:, t % DBUF, :],
                        op0=mybir.AluOpType.mult,
                        op1=mybir.AluOpType.mult,
                        accum_out=acc[:, t : t + 1],
                    ).then_inc(s_dsq, 1)
            # reduce the [1,16] matmul result to the final scalar
            # (clamps run on GpSimd; see the gpsimd block)
            vector.wait_ge(s_mm, 2)
            vector.reduce_sum(
                out=red[:], in_=ps[:], axis=mybir.AxisListType.X
            ).then_inc(s_red, 1)

        @block.scalar
        def _(scalar):
            # idx load on the otherwise-idle ACT HWDGE ring, in parallel with
            # the feature DMAs on the sync ring
            scalar.dma_start(out=idx_sb[:], in_=idx[:]).then_inc(s_idx, 16)
            for t in range(NT):
                if sq_owner[t] != "act":
                    continue
                scalar.wait_ge(s_diff, t + 1)
                scalar.activation(
                    out=dbuf[:, t % DBUF, :],
                    in_=dbuf[:, t % DBUF, :],
                    func=mybir.ActivationFunctionType.Square,
                    accum_out=acc[:, t : t + 1],
                ).then_inc(s_sq, 1)

    return nc


def _make_in_maps(features, labels, centers):
    feats = np.ascontiguousarray(np.asarray(features, dtype=np.float32)).reshape(
        N_CORES, NS, D
    )
    cls = np.asarray(labels)[:, -1].astype(np.int32).reshape(N_CORES, NT, P)
    cent = np.ascontiguousarray(np.asarray(centers, dtype=np.float32))
    in_maps = []
    for i in range(N_CORES):
        in_maps.append(
            {
                "feat": feats[i],
                # idx[p, t] = class id of shard row t*128 + p
                "idx": np.ascontiguousarray(cls[i].T),
                "cent": cent,
            }
        )
    return in_maps


def _run(features, labels, centers, trace=False):
    from concourse.bass_utils import run_bass_kernel_spmd

    if "nc" not in _cache:
        _cache["nc"] = _build()
    in_maps = _make_in_maps(features, labels, centers)
    res = run_bass_kernel_spmd(
        _cache["nc"], in_maps, list(range(N_CORES)), trace=trace
    )
    total = sum(float(r["out"][0, 0]) for r in res.results)
    return np.float32(total / N), res


def kernel(features, labels, centers):
    out, _ = _run(features, labels, centers, trace=False)
    return out

